# revision 4
# baseline (speedup 1.0000x reference)
"""Trainium2 Bass kernel for causal multi-head attention (bf16 v2).

Problem: B=4, S=2048, D=1024, H=16, HD=64, fp32 I/O, causal softmax attention.

Sharding (8 cores): core i handles batch b = i//2 and head-group hg = i%2
(8 of the 16 heads).  Tensor-parallel split: Wq/Wk/Wv columns and Wo rows
are sliced per head-group; each core emits a partial output [S, D] which
the host sums pairwise (the "all-reduce") and adds the output bias.

v2 changes vs the f32r baseline (HW-calibrated: bf16 matmuls stream 2x
faster than float32r on this part; ~190ns/instruction overhead for
sem-waited matmuls):
  - all matmul operands bf16 (hosts casts inputs); psum accumulation f32;
    softmax denominators kept f32 end-to-end.
  - Q^T now stays SBUF-resident (bf16 halves the footprint) - the DRAM
    round-trip and its reload DMAs are gone.
  - attention inner loop is software-pipelined: scores for k-block kb+1
    are emitted before ctx of kb so the in-order PE queue never stalls
    on the ScalarE exp of the current block.
  - diagonal score/ctx matmuls are sliced to exactly the causal width
    (bf16 has no <256 moving-dim rate penalty).
  - (gpsimd/Pool turned out to be unusable for tensor work on this HW:
    no PSUM access and DSP-speed SBUF ops - it only issues DMAs.)
"""

import sys

if "/opt/trn_rl_repo" not in sys.path:
    sys.path.insert(0, "/opt/trn_rl_repo")

from contextlib import ExitStack

import numpy as np

import concourse.bass as bass
import concourse.mybir as mybir
import concourse.tile as tile
from concourse import bacc

F32 = mybir.dt.float32
BF16 = mybir.dt.bfloat16
EXP = mybir.ActivationFunctionType.Exp

# Problem dims (hardcoded per contract).
B, S, D, H, HD = 4, 2048, 1024, 16, 64
N_CORES = 8
HPC = H // (N_CORES // B)  # heads per core = 8
DHC = HPC * HD             # per-core head dims = 512

P = 128    # SBUF partitions
NQ = 512   # q-block width (max matmul moving dim / one PSUM bank)
KBP = NQ // P  # k-blocks per q-block / diagonal offset classes


def build_core_program(S=S, D=D, DHC=DHC, HD=HD, debug=False, reps=1):
    """Build the single-core Bass program (same NEFF runs SPMD on all cores)."""
    nc = bacc.Bacc("TRN2", target_bir_lowering=False, debug=debug)

    xt_d = nc.dram_tensor("xt", [D, S], BF16, kind="ExternalInput").ap()
    wq_d = nc.dram_tensor("wq", [D, DHC], BF16, kind="ExternalInput").ap()
    wk_d = nc.dram_tensor("wk", [D, DHC], BF16, kind="ExternalInput").ap()
    wv_d = nc.dram_tensor("wv", [D, DHC], BF16, kind="ExternalInput").ap()
    wo_d = nc.dram_tensor("wo", [DHC, D], BF16, kind="ExternalInput").ap()
    mask_d = nc.dram_tensor("mask", [KBP, P, NQ], BF16, kind="ExternalInput").ap()
    ones_d = nc.dram_tensor("ones", [P, 128], BF16, kind="ExternalInput").ap()
    out_d = nc.dram_tensor("out", [S, D], F32, kind="ExternalOutput").ap()

    with tile.TileContext(nc) as tc:
        for _ in range(reps):
            _mha_tile_kernel(tc, out_d, xt_d, wq_d, wk_d, wv_d, wo_d, mask_d,
                             ones_d, S=S, D=D, DHC=DHC, HD=HD)
    nc.finalize()
    return nc


def _mha_tile_kernel(tc, out_d, xt_d, wq_d, wk_d, wv_d, wo_d, mask_d,
                     ones_d, *, S, D, DHC, HD):
    nc = tc.nc
    SB = S // NQ        # s-blocks == q-blocks
    PAIRS = DHC // P    # head pairs per core
    OCH = D // P        # contraction chunks for projections
    DOB = D // NQ       # output-dim blocks in out-proj
    QI = NQ // P        # q128-chunks per q-block
    scale = 1.0 / float(np.sqrt(HD))

    ctx = ExitStack()
    with ctx:
        wpool = ctx.enter_context(tc.tile_pool(name="wpool", bufs=1))
        consts = ctx.enter_context(tc.tile_pool(name="consts", bufs=1))
        kv = ctx.enter_context(tc.tile_pool(name="kv", bufs=1))
        xts = ctx.enter_context(tc.tile_pool(name="xts", bufs=1))
        work = ctx.enter_context(tc.tile_pool(name="work", bufs=2))
        psum = ctx.enter_context(tc.tile_pool(name="psum", bufs=1, space="PSUM"))
        dram = ctx.enter_context(tc.tile_pool(name="dram", bufs=1, space="DRAM"))

        # prewarm the ScalarE exp table during the idle startup window
        warm = work.tile([P, 1], F32, tag="warm", bufs=1)
        nc.vector.memset(warm, 1.0)
        nc.scalar.activation(warm, warm, EXP, scale=1.0)

        # --- weights / constants (chunked loads; first matmuls start early) ---
        wq_r = wq_d.rearrange("(o p) m -> p o m", p=P)
        wk_r = wk_d.rearrange("(o p) m -> p o m", p=P)
        wv_r = wv_d.rearrange("(o p) m -> p o m", p=P)
        wq_sb = wpool.tile([P, OCH, DHC], BF16)
        wk_sb = wpool.tile([P, OCH, DHC], BF16)
        wv_sb = wpool.tile([P, OCH, DHC], BF16)
        wo_sb = wpool.tile([P, PAIRS, D], BF16)
        mask_sb = consts.tile([P, KBP, NQ], BF16)

        heads = DHC // HD
        SP = S // P
        # --- persistent Q^T, K^T (head-pair-major) and V+ones ---
        qt2 = kv.tile([P, PAIRS, S], BF16)            # [dh-in-pair, pair, q]
        kt2 = kv.tile([P, PAIRS, S], BF16)            # [dh-in-pair, pair, k]
        vres = kv.tile([P, SP, heads, HD + 1], BF16)  # [s-in, s-out, h, d|1]

        xt_r = xt_d.rearrange("(o p) s -> p o s", p=P)

        scratch = {}

        def outproj_load_pair(ctxt, zb, ctx_dram, z_dram, c):
            # reload + normalize one pair's ctx chunk
            ctx_r = ctx_dram.rearrange("(c p) n -> p c n", p=P)
            nc.sync.dma_start(ctxt[:, c, :], ctx_r[:, c, :])
            for half, h in ((0, 2 * c), (1, 2 * c + 1)):
                z_src = bass.AP(tensor=z_dram.tensor,
                                offset=z_dram.offset + h * NQ,
                                ap=[[0, 64], [1, NQ]])
                nc.sync.dma_start(zb[64 * half:64 * half + 64, c, :], z_src)
            zc = zb[:, c, :]
            nc.vector.reciprocal_approx_fast(out=zc, in_=zc)
            nc.vector.tensor_mul(ctxt[:, c, :], ctxt[:, c, :], zc)

        def outproj_load(j):
            ctx_dram, z_dram = scratch.pop(j)
            ctxt = work.tile([P, PAIRS, NQ], BF16, tag="ctxt", bufs=1)
            zb = work.tile([P, PAIRS, NQ], F32, tag="zb", bufs=1)
            for c in range(PAIRS):
                outproj_load_pair(ctxt, zb, ctx_dram, z_dram, c)
            return ctxt

        def outproj_mms(j, ctxt, qi):
            # one q128-chunk of q-block j's out-projection; emitted between
            # attention pairs as independent PE work
            for nb in range(DOB):
                po = psum.tile([P, NQ], F32, tag="acc", bufs=2)
                for c in range(PAIRS):
                    nc.tensor.matmul(
                        po, lhsT=(ctxt[:, c, qi * P:(qi + 1) * P]),
                        rhs=(wo_sb[:, c, nb * NQ:(nb + 1) * NQ]),
                        start=(c == 0), stop=(c == PAIRS - 1))
                ostage = work.tile([P, NQ], F32, tag="ostage", bufs=3)
                nc.vector.tensor_copy(ostage, po)
                nc.sync.dma_start(
                    out_d[j * NQ + qi * P:j * NQ + (qi + 1) * P,
                          nb * NQ:(nb + 1) * NQ], ostage)

        for t in range(SB):
            # ---- projections for s-block t ----
            xt_sb = xts.tile([P, OCH, NQ], BF16, tag="xt", bufs=1)
            for o in range(OCH):
                if t == 0:
                    nc.scalar.dma_start(wq_sb[:, o, :], wq_r[:, o, :])
                nc.sync.dma_start(xt_sb[:, o, :],
                                  xt_r[:, o, t * NQ:(t + 1) * NQ])
            if t == 0:
                for o in range(OCH):
                    nc.scalar.dma_start(wk_sb[:, o, :], wk_r[:, o, :])
                    nc.gpsimd.dma_start(wv_sb[:, o, :], wv_r[:, o, :])
                nc.sync.dma_start(
                    vres[:, :, :, HD],
                    ones_d[:, 0:SP * heads].rearrange("p (a b) -> p a b", a=SP))
                nc.gpsimd.dma_start(mask_sb, mask_d.rearrange("c p n -> p c n"))
                nc.gpsimd.dma_start(wo_sb, wo_d.rearrange("(c p) n -> p c n", p=P))

            for c in range(PAIRS):
                qps = psum.tile([P, NQ], F32, tag="acc", bufs=2)
                for o in range(OCH):
                    nc.tensor.matmul(
                        qps, lhsT=(wq_sb[:, o, c * P:(c + 1) * P]),
                        rhs=(xt_sb[:, o, :]),
                        start=(o == 0), stop=(o == OCH - 1))
                nc.vector.tensor_copy(qt2[:, c, t * NQ:(t + 1) * NQ], qps)

            for c in range(PAIRS):
                kps = psum.tile([P, NQ], F32, tag="acc", bufs=2)
                for o in range(OCH):
                    nc.tensor.matmul(
                        kps, lhsT=(wk_sb[:, o, c * P:(c + 1) * P]),
                        rhs=(xt_sb[:, o, :]),
                        start=(o == 0), stop=(o == OCH - 1))
                nc.vector.tensor_copy(kt2[:, c, t * NQ:(t + 1) * NQ], kps)

            for i in range(KBP):
                vps = psum.tile([P, DHC], F32, tag="acc", bufs=2)
                for o in range(OCH):
                    nc.tensor.matmul(
                        vps, lhsT=(xt_sb[:, o, i * P:(i + 1) * P]),
                        rhs=(wv_sb[:, o, :]),
                        start=(o == 0), stop=(o == OCH - 1))
                nc.vector.tensor_copy(
                    vres[:, t * KBP + i, :, 0:HD],
                    vps.rearrange("p (h d) -> p h d", d=HD))

            # out-proj inputs of the previous q-block
            prev_ctxt = outproj_load(t - 1) if t >= 1 else None

            # ---- attention for q-block j = t (causal: s-blocks <= t) ----
            j = t
            ctx_dram = dram.tile([DHC, NQ], BF16, tag="ctxd", bufs=2)
            z_dram = dram.tile([heads, NQ], F32, tag="zd", bufs=2)
            scratch[j] = (ctx_dram, z_dram)
            if t == SB - 1:
                last_ctxt = work.tile([P, PAIRS, NQ], BF16, tag="ctxt", bufs=1)
                last_zb = work.tile([P, PAIRS, NQ], F32, tag="zb", bufs=1)
            for c in range(PAIRS):
                qp0 = qt2[0:64, c, j * NQ:(j + 1) * NQ]
                qp1 = qt2[64:128, c, j * NQ:(j + 1) * NQ]
                cx_e = psum.tile([HD + 1, NQ], F32, tag="cxe", bufs=1)
                cx_o = psum.tile([HD + 1, NQ], F32, tag="cxo", bufs=1)
                KB = (j + 1) * KBP

                pend = []  # software-pipeline: ctx of kb runs behind scores of kb+1

                def emit_scores(kb):
                    d = kb * P - j * NQ  # >= 0 on the causal diagonal band
                    lo = max(d, 0)
                    st = psum.tile([P, 2, NQ], F32, tag="st", bufs=2)
                    nc.tensor.matmul(
                        st[:, 0, lo:NQ],
                        lhsT=(kt2[0:64, c, kb * P:(kb + 1) * P]),
                        rhs=(qp0[:, lo:NQ]), start=True, stop=True)
                    nc.tensor.matmul(
                        st[:, 1, lo:NQ],
                        lhsT=(kt2[64:128, c, kb * P:(kb + 1) * P]),
                        rhs=(qp1[:, lo:NQ]), start=True, stop=True)
                    ex = work.tile([P, 2, NQ], BF16, tag="ex", bufs=3)
                    nc.scalar.activation(ex[:, :, lo:NQ], st[:, :, lo:NQ],
                                         EXP, scale=scale)
                    if d >= 0:
                        nc.vector.tensor_mul(
                            ex[:, :, d:d + P], ex[:, :, d:d + P],
                            mask_sb[:, d // P, None, d:d + P]
                            .to_broadcast([P, 2, P]))
                    pend.append((kb, lo, ex))

                def emit_ctx():
                    kb, lo, ex = pend.pop(0)
                    first, last = (kb == 0), (kb == KB - 1)
                    nc.tensor.matmul(
                        cx_e[:, lo:NQ], lhsT=(vres[:, kb, 2 * c, :]),
                        rhs=(ex[:, 0, lo:NQ]), start=first, stop=last,
                        skip_group_check=True)
                    nc.tensor.matmul(
                        cx_o[:, lo:NQ], lhsT=(vres[:, kb, 2 * c + 1, :]),
                        rhs=(ex[:, 1, lo:NQ]), start=first, stop=last,
                        skip_group_check=True)

                for kb in range(KB):
                    emit_scores(kb)
                    if len(pend) > 1:
                        emit_ctx()
                while pend:
                    emit_ctx()

                # spill unnormalized ctx rows (bf16) and the Z row (f32)
                for h, cx in ((2 * c, cx_e), (2 * c + 1, cx_o)):
                    cst = work.tile([HD, NQ], BF16, tag="cst", bufs=2)
                    zst = work.tile([1, NQ], F32, tag="zst", bufs=2)
                    nc.vector.tensor_copy(cst, cx[0:HD, :])
                    nc.vector.tensor_copy(zst, cx[HD:HD + 1, :])
                    nc.sync.dma_start(ctx_dram[h * HD:(h + 1) * HD, :], cst)
                    nc.sync.dma_start(z_dram[h:h + 1, :], zst)
                if prev_ctxt is not None:
                    for qi in range(c * QI // PAIRS, (c + 1) * QI // PAIRS):
                        outproj_mms(t - 1, prev_ctxt, qi)
                if t == SB - 1:
                    outproj_load_pair(last_ctxt, last_zb, ctx_dram, z_dram, c)

        scratch.pop(SB - 1)
        for qi in range(QI):
            outproj_mms(SB - 1, last_ctxt, qi)


_MASK = np.stack([
    (np.arange(P)[:, None] + c * P <= np.arange(NQ)[None, :])
    for c in range(KBP)
]).astype(np.float32)
_ONES = np.ones((P, 128), np.float32)

_PROGRAM_CACHE = {}


def _get_program():
    if "nc" not in _PROGRAM_CACHE:
        _PROGRAM_CACHE["nc"] = build_core_program()
    return _PROGRAM_CACHE["nc"]


def _get_runner():
    """Sharded jitted callable over the 8 cores."""
    if "runner" in _PROGRAM_CACHE:
        return _PROGRAM_CACHE["runner"]

    import jax
    from jax.sharding import Mesh, PartitionSpec
    from jax.experimental.shard_map import shard_map
    import concourse.mybir as _mybir
    from concourse import bass2jax

    nc = _get_program()
    bass2jax.install_neuronx_cc_hook()

    partition_name = (nc.partition_id_tensor.name
                      if nc.partition_id_tensor else None)
    in_names, out_names, out_avals, zero_outs = [], [], [], []
    for alloc in nc.m.functions[0].allocations:
        if not isinstance(alloc, _mybir.MemoryLocationSet):
            continue
        name = alloc.memorylocations[0].name
        if alloc.kind == "ExternalInput":
            if name != partition_name:
                in_names.append(name)
        elif alloc.kind == "ExternalOutput":
            out_names.append(name)
            shape = tuple(alloc.tensor_shape)
            dtype = _mybir.dt.np(alloc.dtype)
            out_avals.append(jax.core.ShapedArray(shape, dtype))
            zero_outs.append(np.zeros(shape, dtype))
    n_params = len(in_names)
    all_names = in_names + out_names
    if partition_name is not None:
        all_names = all_names + [partition_name]

    def _body(*args):
        operands = list(args)
        if partition_name is not None:
            operands.append(bass2jax.partition_id_tensor())
        outs = bass2jax._bass_exec_p.bind(
            *operands,
            out_avals=tuple(out_avals),
            in_names=tuple(all_names),
            out_names=tuple(out_names),
            lowering_input_output_aliases=(),
            sim_require_finite=True,
            sim_require_nnan=True,
            nc=nc,
        )
        return tuple(outs)

    devices = jax.devices()[:N_CORES]
    mesh = Mesh(np.asarray(devices), ("core",))
    n_args = n_params + len(out_names)
    sharded = jax.jit(
        shard_map(_body, mesh=mesh,
                  in_specs=(PartitionSpec("core"),) * n_args,
                  out_specs=(PartitionSpec("core"),) * len(out_names),
                  check_rep=False),
        keep_unused=True,
    )
    runner = dict(fn=sharded, in_names=in_names, out_names=out_names,
                  out_avals=out_avals, zero_outs=zero_outs, mesh=mesh)
    _PROGRAM_CACHE["runner"] = runner
    return runner


def run_on_cores(in_maps, runner=None):
    """Execute the SPMD program; returns list of per-core output dicts."""
    runner = runner or _get_runner()
    concat_in = [
        np.concatenate([np.asarray(in_maps[c][name]) for c in range(N_CORES)],
                       axis=0)
        for name in runner["in_names"]
    ]
    concat_zeros = [
        np.zeros((N_CORES * z.shape[0], *z.shape[1:]), z.dtype)
        for z in runner["zero_outs"]
    ]
    out_arrs = runner["fn"](*concat_in, *concat_zeros)
    return [
        {name: np.asarray(out_arrs[i]).reshape(N_CORES, *runner["out_avals"][i].shape)[c]
         for i, name in enumerate(runner["out_names"])}
        for c in range(N_CORES)
    ]


def make_in_maps(x, Wq, Wk, Wv, Wo):
    import ml_dtypes
    bf16 = ml_dtypes.bfloat16
    in_maps = []
    for core in range(N_CORES):
        b, hg = divmod(core, 2)
        sl = slice(hg * DHC, (hg + 1) * DHC)
        in_maps.append({
            "xt": np.ascontiguousarray(x[b].T).astype(bf16),
            "wq": np.ascontiguousarray(Wq[:, sl]).astype(bf16),
            "wk": np.ascontiguousarray(Wk[:, sl]).astype(bf16),
            "wv": np.ascontiguousarray(Wv[:, sl]).astype(bf16),
            "wo": np.ascontiguousarray(Wo[sl, :]).astype(bf16),
            "mask": _MASK.astype(bf16),
            "ones": _ONES.astype(bf16),
        })
    return in_maps


def kernel(x, Wq, Wk, Wv, Wo, bo, _collect=None):
    x = np.asarray(x, dtype=np.float32)
    Wq = np.asarray(Wq, dtype=np.float32)
    Wk = np.asarray(Wk, dtype=np.float32)
    Wv = np.asarray(Wv, dtype=np.float32)
    Wo = np.asarray(Wo, dtype=np.float32)
    bo = np.asarray(bo, dtype=np.float32)

    in_maps = make_in_maps(x, Wq, Wk, Wv, Wo)
    results = run_on_cores(in_maps)
    if _collect is not None:
        _collect.append(results)

    outs = [r["out"] for r in results]
    out = np.empty((B, S, D), np.float32)
    for b in range(B):
        out[b] = outs[2 * b] + outs[2 * b + 1] + bo
    return out


# revision 6
# speedup vs baseline: 1.2620x; 1.2620x over previous
"""Trainium2 Bass kernel for causal multi-head attention (bf16 v2).

Problem: B=4, S=2048, D=1024, H=16, HD=64, fp32 I/O, causal softmax attention.

Sharding (8 cores): core i handles batch b = i//2 and head-group hg = i%2
(8 of the 16 heads).  Tensor-parallel split: Wq/Wk/Wv columns and Wo rows
are sliced per head-group; each core emits a partial output [S, D] which
the host sums pairwise (the "all-reduce") and adds the output bias.

v2 changes vs the f32r baseline (HW-calibrated: bf16 matmuls stream 2x
faster than float32r on this part; ~190ns/instruction overhead for
sem-waited matmuls):
  - all matmul operands bf16 (hosts casts inputs); psum accumulation f32;
    softmax denominators kept f32 end-to-end.
  - Q^T now stays SBUF-resident (bf16 halves the footprint) - the DRAM
    round-trip and its reload DMAs are gone.
  - attention inner loop is software-pipelined: scores for k-block kb+1
    are emitted before ctx of kb so the in-order PE queue never stalls
    on the ScalarE exp of the current block.
  - diagonal score/ctx matmuls are sliced to exactly the causal width
    (bf16 has no <256 moving-dim rate penalty).
  - (gpsimd/Pool turned out to be unusable for tensor work on this HW:
    no PSUM access and DSP-speed SBUF ops - it only issues DMAs.)
"""

import sys

if "/opt/trn_rl_repo" not in sys.path:
    sys.path.insert(0, "/opt/trn_rl_repo")

from contextlib import ExitStack

import numpy as np

import concourse.bass as bass
import concourse.mybir as mybir
import concourse.tile as tile
from concourse import bacc

F32 = mybir.dt.float32
BF16 = mybir.dt.bfloat16
EXP = mybir.ActivationFunctionType.Exp

# Problem dims (hardcoded per contract).
B, S, D, H, HD = 4, 2048, 1024, 16, 64
N_CORES = 8
HPC = H // (N_CORES // B)  # heads per core = 8
DHC = HPC * HD             # per-core head dims = 512

P = 128    # SBUF partitions
NQ = 512   # q-block width (max matmul moving dim / one PSUM bank)
KBP = NQ // P  # k-blocks per q-block / diagonal offset classes


def build_core_program(S=S, D=D, DHC=DHC, HD=HD, debug=False, reps=1):
    """Build the single-core Bass program (same NEFF runs SPMD on all cores)."""
    nc = bacc.Bacc("TRN2", target_bir_lowering=False, debug=debug)

    xt_d = nc.dram_tensor("xt", [D, S], BF16, kind="ExternalInput").ap()
    wq_d = nc.dram_tensor("wq", [D, DHC], BF16, kind="ExternalInput").ap()
    wk_d = nc.dram_tensor("wk", [D, DHC], BF16, kind="ExternalInput").ap()
    wv_d = nc.dram_tensor("wv", [D, DHC], BF16, kind="ExternalInput").ap()
    wo_d = nc.dram_tensor("wo", [DHC, D], BF16, kind="ExternalInput").ap()
    mask_d = nc.dram_tensor("mask", [KBP, P, NQ], BF16, kind="ExternalInput").ap()
    ones_d = nc.dram_tensor("ones", [P, 128], BF16, kind="ExternalInput").ap()
    out_d = nc.dram_tensor("out", [S, D], F32, kind="ExternalOutput").ap()

    with tile.TileContext(nc) as tc:
        for _ in range(reps):
            _mha_tile_kernel(tc, out_d, xt_d, wq_d, wk_d, wv_d, wo_d, mask_d,
                             ones_d, S=S, D=D, DHC=DHC, HD=HD)
    nc.finalize()
    return nc


def _mha_tile_kernel(tc, out_d, xt_d, wq_d, wk_d, wv_d, wo_d, mask_d,
                     ones_d, *, S, D, DHC, HD):
    nc = tc.nc
    SB = S // NQ        # s-blocks == q-blocks
    PAIRS = DHC // P    # head pairs per core
    OCH = D // P        # contraction chunks for projections
    DOB = D // NQ       # output-dim blocks in out-proj
    QI = NQ // P        # q128-chunks per q-block
    scale = 1.0 / float(np.sqrt(HD))

    ctx = ExitStack()
    with ctx:
        wpool = ctx.enter_context(tc.tile_pool(name="wpool", bufs=1))
        consts = ctx.enter_context(tc.tile_pool(name="consts", bufs=1))
        kv = ctx.enter_context(tc.tile_pool(name="kv", bufs=1))
        xts = ctx.enter_context(tc.tile_pool(name="xts", bufs=1))
        work = ctx.enter_context(tc.tile_pool(name="work", bufs=2))
        psum = ctx.enter_context(tc.tile_pool(name="psum", bufs=1, space="PSUM"))
        dram = ctx.enter_context(tc.tile_pool(name="dram", bufs=1, space="DRAM"))

        # prewarm the ScalarE exp table during the idle startup window
        warm = work.tile([P, 1], F32, tag="warm", bufs=1)
        nc.vector.memset(warm, 1.0)
        nc.scalar.activation(warm, warm, EXP, scale=1.0)

        # --- weights / constants (chunked loads; first matmuls start early) ---
        wq_r = wq_d.rearrange("(o p) m -> p o m", p=P)
        wk_r = wk_d.rearrange("(o p) m -> p o m", p=P)
        wv_r = wv_d.rearrange("(o p) m -> p o m", p=P)
        wq_sb = wpool.tile([P, OCH, DHC], BF16)
        wk_sb = wpool.tile([P, OCH, DHC], BF16)
        wv_sb = wpool.tile([P, OCH, DHC], BF16)
        wo_sb = wpool.tile([P, PAIRS, D], BF16)
        mask_sb = consts.tile([P, KBP, NQ], BF16)

        heads = DHC // HD
        SP = S // P
        # --- persistent Q^T, K^T (head-pair-major) and V+ones ---
        qt2 = kv.tile([P, PAIRS, S], BF16)            # [dh-in-pair, pair, q]
        kt2 = kv.tile([P, PAIRS, S], BF16)            # [dh-in-pair, pair, k]
        vres = kv.tile([P, SP, heads, HD + 1], BF16)  # [s-in, s-out, h, d|1]

        xt_r = xt_d.rearrange("(o p) s -> p o s", p=P)

        scratch = {}

        def outproj_load_pair(ctxt, zb, ctx_dram, z_dram, c):
            # reload + normalize one pair's ctx chunk
            ctx_r = ctx_dram.rearrange("(c p) n -> p c n", p=P)
            nc.sync.dma_start(ctxt[:, c, :], ctx_r[:, c, :])
            for half, h in ((0, 2 * c), (1, 2 * c + 1)):
                z_src = bass.AP(tensor=z_dram.tensor,
                                offset=z_dram.offset + h * NQ,
                                ap=[[0, 64], [1, NQ]])
                nc.sync.dma_start(zb[64 * half:64 * half + 64, c, :], z_src)
            zc = zb[:, c, :]
            nc.vector.reciprocal_approx_fast(out=zc, in_=zc)
            nc.vector.tensor_mul(ctxt[:, c, :], ctxt[:, c, :], zc)

        def outproj_load(j):
            ctx_dram, z_dram = scratch.pop(j)
            ctxt = work.tile([P, PAIRS, NQ], BF16, tag="ctxt", bufs=1)
            zb = work.tile([P, PAIRS, NQ], F32, tag="zb", bufs=1)
            for c in range(PAIRS):
                outproj_load_pair(ctxt, zb, ctx_dram, z_dram, c)
            return ctxt

        def outproj_mms(j, ctxt, qi):
            # one q128-chunk of q-block j's out-projection; emitted between
            # attention pairs as independent PE work
            for nb in range(DOB):
                po = psum.tile([P, NQ], F32, tag="acc", bufs=2)
                for c in range(PAIRS):
                    nc.tensor.matmul(
                        po, lhsT=(ctxt[:, c, qi * P:(qi + 1) * P]),
                        rhs=(wo_sb[:, c, nb * NQ:(nb + 1) * NQ]),
                        start=(c == 0), stop=(c == PAIRS - 1))
                ostage = work.tile([P, NQ], F32, tag="ostage", bufs=3)
                nc.vector.tensor_copy(ostage, po)
                nc.sync.dma_start(
                    out_d[j * NQ + qi * P:j * NQ + (qi + 1) * P,
                          nb * NQ:(nb + 1) * NQ], ostage)

        for t in range(SB):
            # ---- projections for s-block t ----
            xt_sb = xts.tile([P, OCH, NQ], BF16, tag="xt", bufs=1)
            for o in range(OCH):
                if t == 0:
                    nc.scalar.dma_start(wq_sb[:, o, :], wq_r[:, o, :])
                nc.sync.dma_start(xt_sb[:, o, :],
                                  xt_r[:, o, t * NQ:(t + 1) * NQ])
            if t == 0:
                for o in range(OCH):
                    nc.scalar.dma_start(wk_sb[:, o, :], wk_r[:, o, :])
                    nc.gpsimd.dma_start(wv_sb[:, o, :], wv_r[:, o, :])
                nc.sync.dma_start(
                    vres[:, :, :, HD],
                    ones_d[:, 0:SP * heads].rearrange("p (a b) -> p a b", a=SP))
                nc.gpsimd.dma_start(mask_sb, mask_d.rearrange("c p n -> p c n"))
                nc.gpsimd.dma_start(wo_sb, wo_d.rearrange("(c p) n -> p c n", p=P))

            for c in range(PAIRS):
                qps = psum.tile([P, NQ], F32, tag="acc", bufs=2)
                for o in range(OCH):
                    nc.tensor.matmul(
                        qps, lhsT=(wq_sb[:, o, c * P:(c + 1) * P]),
                        rhs=(xt_sb[:, o, :]),
                        start=(o == 0), stop=(o == OCH - 1))
                nc.vector.tensor_copy(qt2[:, c, t * NQ:(t + 1) * NQ], qps)

            for c in range(PAIRS):
                kps = psum.tile([P, NQ], F32, tag="acc", bufs=2)
                for o in range(OCH):
                    nc.tensor.matmul(
                        kps, lhsT=(wk_sb[:, o, c * P:(c + 1) * P]),
                        rhs=(xt_sb[:, o, :]),
                        start=(o == 0), stop=(o == OCH - 1))
                nc.vector.tensor_copy(kt2[:, c, t * NQ:(t + 1) * NQ], kps)

            for i in range(KBP):
                vps = psum.tile([P, DHC], F32, tag="acc", bufs=2)
                for o in range(OCH):
                    nc.tensor.matmul(
                        vps, lhsT=(xt_sb[:, o, i * P:(i + 1) * P]),
                        rhs=(wv_sb[:, o, :]),
                        start=(o == 0), stop=(o == OCH - 1))
                nc.vector.tensor_copy(
                    vres[:, t * KBP + i, :, 0:HD],
                    vps.rearrange("p (h d) -> p h d", d=HD))

            # out-proj inputs of the previous q-block
            prev_ctxt = outproj_load(t - 1) if t >= 1 else None

            # ---- attention for q-block j = t (causal: s-blocks <= t) ----
            j = t
            ctx_dram = dram.tile([DHC, NQ], BF16, tag="ctxd", bufs=2)
            z_dram = dram.tile([heads, NQ], F32, tag="zd", bufs=2)
            scratch[j] = (ctx_dram, z_dram)
            if t == SB - 1:
                last_ctxt = work.tile([P, PAIRS, NQ], BF16, tag="ctxt", bufs=1)
                last_zb = work.tile([P, PAIRS, NQ], F32, tag="zb", bufs=1)
            for c in range(PAIRS):
                qp0 = qt2[0:64, c, j * NQ:(j + 1) * NQ]
                qp1 = qt2[64:128, c, j * NQ:(j + 1) * NQ]
                cx_e = psum.tile([HD + 1, NQ], F32, tag="cxe", bufs=1)
                cx_o = psum.tile([HD + 1, NQ], F32, tag="cxo", bufs=1)
                KB = (j + 1) * KBP

                pend = []  # software-pipeline: ctx of kb runs behind scores of kb+1

                def emit_scores(kb):
                    d = kb * P - j * NQ  # >= 0 on the causal diagonal band
                    lo = max(d, 0)
                    st = psum.tile([P, 2, NQ], F32, tag="st", bufs=2)
                    nc.tensor.matmul(
                        st[:, 0, lo:NQ],
                        lhsT=(kt2[0:64, c, kb * P:(kb + 1) * P]),
                        rhs=(qp0[:, lo:NQ]), start=True, stop=True)
                    nc.tensor.matmul(
                        st[:, 1, lo:NQ],
                        lhsT=(kt2[64:128, c, kb * P:(kb + 1) * P]),
                        rhs=(qp1[:, lo:NQ]), start=True, stop=True)
                    ex = work.tile([P, 2, NQ], BF16, tag="ex", bufs=3)
                    nc.scalar.activation(ex[:, :, lo:NQ], st[:, :, lo:NQ],
                                         EXP, scale=scale)
                    if d >= 0:
                        nc.vector.tensor_mul(
                            ex[:, :, d:d + P], ex[:, :, d:d + P],
                            mask_sb[:, d // P, None, d:d + P]
                            .to_broadcast([P, 2, P]))
                    pend.append((kb, lo, ex))

                def emit_ctx():
                    kb, lo, ex = pend.pop(0)
                    first, last = (kb == 0), (kb == KB - 1)
                    nc.tensor.matmul(
                        cx_e[:, lo:NQ], lhsT=(vres[:, kb, 2 * c, :]),
                        rhs=(ex[:, 0, lo:NQ]), start=first, stop=last,
                        skip_group_check=True)
                    nc.tensor.matmul(
                        cx_o[:, lo:NQ], lhsT=(vres[:, kb, 2 * c + 1, :]),
                        rhs=(ex[:, 1, lo:NQ]), start=first, stop=last,
                        skip_group_check=True)

                for kb in range(KB):
                    emit_scores(kb)
                    if len(pend) > 1:
                        emit_ctx()
                while pend:
                    emit_ctx()

                # spill unnormalized ctx rows (bf16) and the Z row (f32)
                for h, cx in ((2 * c, cx_e), (2 * c + 1, cx_o)):
                    cst = work.tile([HD, NQ], BF16, tag="cst", bufs=2)
                    zst = work.tile([1, NQ], F32, tag="zst", bufs=2)
                    nc.vector.tensor_copy(cst, cx[0:HD, :])
                    nc.vector.tensor_copy(zst, cx[HD:HD + 1, :])
                    nc.sync.dma_start(ctx_dram[h * HD:(h + 1) * HD, :], cst)
                    nc.sync.dma_start(z_dram[h:h + 1, :], zst)
                if prev_ctxt is not None:
                    for qi in range(c * QI // PAIRS, (c + 1) * QI // PAIRS):
                        outproj_mms(t - 1, prev_ctxt, qi)
                if t == SB - 1:
                    outproj_load_pair(last_ctxt, last_zb, ctx_dram, z_dram, c)

        scratch.pop(SB - 1)
        for qi in range(QI):
            outproj_mms(SB - 1, last_ctxt, qi)


_MASK = np.stack([
    (np.arange(P)[:, None] + c * P <= np.arange(NQ)[None, :])
    for c in range(KBP)
]).astype(np.float32)
_ONES = np.ones((P, 128), np.float32)

_PROGRAM_CACHE = {}


def _get_program():
    if "nc" not in _PROGRAM_CACHE:
        _PROGRAM_CACHE["nc"] = build_core_program()
    return _PROGRAM_CACHE["nc"]


def _get_runner():
    """Sharded jitted callable over the 8 cores."""
    if "runner" in _PROGRAM_CACHE:
        return _PROGRAM_CACHE["runner"]

    import jax
    from jax.sharding import Mesh, PartitionSpec
    from jax.experimental.shard_map import shard_map
    import concourse.mybir as _mybir
    from concourse import bass2jax

    nc = _get_program()
    bass2jax.install_neuronx_cc_hook()

    partition_name = (nc.partition_id_tensor.name
                      if nc.partition_id_tensor else None)
    in_names, out_names, out_avals, zero_outs = [], [], [], []
    for alloc in nc.m.functions[0].allocations:
        if not isinstance(alloc, _mybir.MemoryLocationSet):
            continue
        name = alloc.memorylocations[0].name
        if alloc.kind == "ExternalInput":
            if name != partition_name:
                in_names.append(name)
        elif alloc.kind == "ExternalOutput":
            out_names.append(name)
            shape = tuple(alloc.tensor_shape)
            dtype = _mybir.dt.np(alloc.dtype)
            out_avals.append(jax.core.ShapedArray(shape, dtype))
            zero_outs.append(np.zeros(shape, dtype))
    n_params = len(in_names)
    all_names = in_names + out_names
    if partition_name is not None:
        all_names = all_names + [partition_name]

    def _body(*args):
        operands = list(args)
        if partition_name is not None:
            operands.append(bass2jax.partition_id_tensor())
        outs = bass2jax._bass_exec_p.bind(
            *operands,
            out_avals=tuple(out_avals),
            in_names=tuple(all_names),
            out_names=tuple(out_names),
            lowering_input_output_aliases=(),
            sim_require_finite=True,
            sim_require_nnan=True,
            nc=nc,
        )
        return tuple(outs)

    devices = jax.devices()[:N_CORES]
    mesh = Mesh(np.asarray(devices), ("core",))
    n_args = n_params + len(out_names)
    sharded = jax.jit(
        shard_map(_body, mesh=mesh,
                  in_specs=(PartitionSpec("core"),) * n_args,
                  out_specs=(PartitionSpec("core"),) * len(out_names),
                  check_rep=False),
        keep_unused=True,
    )
    runner = dict(fn=sharded, in_names=in_names, out_names=out_names,
                  out_avals=out_avals, zero_outs=zero_outs, mesh=mesh)
    _PROGRAM_CACHE["runner"] = runner
    return runner


def run_on_cores(in_maps, runner=None):
    """Execute the SPMD program; returns list of per-core output dicts."""
    runner = runner or _get_runner()
    concat_in = [
        np.concatenate([np.asarray(in_maps[c][name]) for c in range(N_CORES)],
                       axis=0)
        for name in runner["in_names"]
    ]
    concat_zeros = [
        np.zeros((N_CORES * z.shape[0], *z.shape[1:]), z.dtype)
        for z in runner["zero_outs"]
    ]
    out_arrs = runner["fn"](*concat_in, *concat_zeros)
    return [
        {name: np.asarray(out_arrs[i]).reshape(N_CORES, *runner["out_avals"][i].shape)[c]
         for i, name in enumerate(runner["out_names"])}
        for c in range(N_CORES)
    ]


def make_in_maps(x, Wq, Wk, Wv, Wo):
    import ml_dtypes
    bf16 = ml_dtypes.bfloat16
    in_maps = []
    for core in range(N_CORES):
        b, hg = divmod(core, 2)
        sl = slice(hg * DHC, (hg + 1) * DHC)
        in_maps.append({
            "xt": np.ascontiguousarray(x[b].T).astype(bf16),
            "wq": np.ascontiguousarray(Wq[:, sl]).astype(bf16),
            "wk": np.ascontiguousarray(Wk[:, sl]).astype(bf16),
            "wv": np.ascontiguousarray(Wv[:, sl]).astype(bf16),
            "wo": np.ascontiguousarray(Wo[sl, :]).astype(bf16),
            "mask": _MASK.astype(bf16),
            "ones": _ONES.astype(bf16),
        })
    return in_maps


def kernel(x, Wq, Wk, Wv, Wo, bo, _collect=None):
    x = np.asarray(x, dtype=np.float32)
    Wq = np.asarray(Wq, dtype=np.float32)
    Wk = np.asarray(Wk, dtype=np.float32)
    Wv = np.asarray(Wv, dtype=np.float32)
    Wo = np.asarray(Wo, dtype=np.float32)
    bo = np.asarray(bo, dtype=np.float32)

    in_maps = make_in_maps(x, Wq, Wk, Wv, Wo)
    results = run_on_cores(in_maps)
    if _collect is not None:
        _collect.append(results)

    outs = [r["out"] for r in results]
    out = np.empty((B, S, D), np.float32)
    for b in range(B):
        out[b] = outs[2 * b] + outs[2 * b + 1] + bo
    return out
